# revision 28
# baseline (speedup 1.0000x reference)
"""Differential self-attention on 8 Trainium2 NeuronCores.

Sharding: batch x head-group. Core c handles batch b = c//4 and heads
hs = 4*(c%4) .. 4*(c%4)+4 (4 of 16 heads). Each core computes q/k/v
projections for its heads, RoPE, both causal softmax score matrices
(flash-style, unnormalized, row sums fused into the attn@v matmul via a
ones-column appended to V), the differential combination + RMS norm
(quake rsqrt on DVE), and a partial out-projection over its heads'
dims. Host sums the 4 partial y per batch and adds bo.

Per-core layouts (feature-major "T" = [feat, seq]):
  qT/kT  [128, 4, 2048]  rows 0:64 = component 1, 64:128 = component 2
  v      [128, 16, 4, 65] (key-block, head, hd + ones col for row sums)
  scores sT [j=128, i<=512] so attn@v contracts j on partitions.

Row broadcasts (1/r, rsqrt(ms)) are PE matmuls with ones/(-lambda)
lhsT; no DRAM round-trips, no Ln activations (single act table load).
"""

import math

import numpy as np
import ml_dtypes

B, S, D = 2, 2048, 1024
H, HD = 16, 64
HALF = HD // 2
NCORES = 8
NH = 4            # heads per core
LAMBDA_INIT = 0.2
EPS = 1e-6
CH = 512          # query chunk
NCH = S // CH
JB = 128          # key block
BF16 = ml_dtypes.bfloat16

_cache = {}


def _build():
    import concourse.bass as bass
    import concourse.tile as tile
    from concourse import bacc, mybir

    F32R = mybir.dt.float32r
    F32 = mybir.dt.float32
    BF = mybir.dt.bfloat16
    U32 = mybir.dt.uint32
    AF = mybir.ActivationFunctionType
    ALU = mybir.AluOpType

    nc = bacc.Bacc("TRN2", debug=False, num_devices=NCORES)

    xT = nc.dram_tensor("xT", [D, S], F32R, kind="ExternalInput")
    wqT = nc.dram_tensor("wqT", [128, 8, 512], F32R, kind="ExternalInput")
    wkT = nc.dram_tensor("wkT", [128, 8, 512], F32R, kind="ExternalInput")
    wvT = nc.dram_tensor("wvT", [128, 8, 256], F32R, kind="ExternalInput")
    bqp = nc.dram_tensor("bqp", [128, 8], F32, kind="ExternalInput")
    bv = nc.dram_tensor("bv", [1, 256], F32R, kind="ExternalInput")
    woT = nc.dram_tensor("woT", [128, 2, 1024], BF, kind="ExternalInput")
    ropeA = nc.dram_tensor("ropeA", [128, S], BF, kind="ExternalInput")
    ropeB = nc.dram_tensor("ropeB", [128, S], BF, kind="ExternalInput")
    mask2 = nc.dram_tensor("mask2", [128, 256], BF, kind="ExternalInput")
    pmat = nc.dram_tensor("pmat", [128, 128], BF, kind="ExternalInput")
    bc1 = nc.dram_tensor("bc1", [128, 320], BF, kind="ExternalInput")
    onesd = nc.dram_tensor("onesd", [1, 512], F32R, kind="ExternalInput")
    y_out = nc.dram_tensor("y", [S, D], F32, kind="ExternalOutput")
    DBG = False
    if DBG:
        dbg_q = nc.dram_tensor("dbg_q", [128, NH * S], mybir.dt.bfloat16,
                               kind="ExternalOutput")
        dbg_k = nc.dram_tensor("dbg_k", [128, NH * S], mybir.dt.bfloat16,
                               kind="ExternalOutput")
        dbg_v = nc.dram_tensor("dbg_v", [128, 16 * NH * 65], mybir.dt.bfloat16,
                               kind="ExternalOutput")
        dbg_occ = nc.dram_tensor("dbg_occ", [128, 1024], F32,
                                 kind="ExternalOutput")
        dbg_y3 = nc.dram_tensor("dbg_y3", [128, 512], mybir.dt.bfloat16,
                                kind="ExternalOutput")
        dbg_inv = nc.dram_tensor("dbg_inv", [1, 1024], mybir.dt.bfloat16,
                                 kind="ExternalOutput")

    xT_r = xT.ap().rearrange("(t p) s -> p t s", p=128)
    QMAGIC = (0x5F3759DF * 2 + 1) & 0xFFFFFFFF

    with tile.TileContext(nc) as tc:
        import contextlib
        ctx = contextlib.ExitStack()
        with ctx:
            persist = ctx.enter_context(tc.tile_pool(name="persist", bufs=1))
            xpool = ctx.enter_context(tc.tile_pool(name="xc", bufs=2))
            rpool = ctx.enter_context(tc.tile_pool(name="rope", bufs=2))
            apool = ctx.enter_context(tc.tile_pool(name="atile", bufs=4))
            npool = ctx.enter_context(tc.tile_pool(name="norm", bufs=1))
            ypool = ctx.enter_context(tc.tile_pool(name="y", bufs=2))
            psum = ctx.enter_context(tc.tile_pool(name="ps", bufs=2, space="PSUM"))
            psc = ctx.enter_context(tc.tile_pool(name="psc", bufs=4, space="PSUM"))

            # ---- persistent tiles (DMA order: first-proj deps first)
            wq_sb = persist.tile([128, 8, 512], F32R, tag="wq")
            nc.sync.dma_start(out=wq_sb[:, 0:1, :], in_=wqT.ap()[:, 0:1, :])
            bqp_sb = persist.tile([128, 8], F32, tag="bqp")
            nc.sync.dma_start(out=bqp_sb[:], in_=bqp.ap())
            xc0 = xpool.tile([128, 8, CH], F32R, tag="xc", name="xc_pre0")
            nc.sync.dma_start(out=xc0[:, 0:1, :], in_=xT_r[:, 0:1, 0:CH])
            for hseg in range(1, 4):
                s0, s1 = 2 * hseg, 2 * hseg + 2
                nc.sync.dma_start(out=wq_sb[:, s0 - 1:s1 - 1, :],
                                  in_=wqT.ap()[:, s0 - 1:s1 - 1, :])
                nc.sync.dma_start(out=xc0[:, s0 - 1:s1 - 1, :],
                                  in_=xT_r[:, s0 - 1:s1 - 1, 0:CH])
            nc.sync.dma_start(out=wq_sb[:, 7:8, :], in_=wqT.ap()[:, 7:8, :])
            nc.sync.dma_start(out=xc0[:, 7:8, :], in_=xT_r[:, 7:8, 0:CH])
            rA_sb = persist.tile([128, S], BF, tag="ra")
            nc.sync.dma_start(out=rA_sb[:], in_=ropeA.ap())
            rB_sb = persist.tile([128, S], BF, tag="rb")
            nc.sync.dma_start(out=rB_sb[:], in_=ropeB.ap())
            pm_sb = persist.tile([128, 128], BF, tag="pm")
            nc.sync.dma_start(out=pm_sb[:], in_=pmat.ap())
            wk_sb = persist.tile([128, 8, 512], F32R, tag="wk")
            nc.sync.dma_start(out=wk_sb[:], in_=wkT.ap())
            wv_sb = persist.tile([128, 8, 256], F32R, tag="wv")
            nc.sync.dma_start(out=wv_sb[:], in_=wvT.ap())
            bv_sb = persist.tile([1, 256], F32R, tag="bv")
            nc.sync.dma_start(out=bv_sb[:], in_=bv.ap())
            onesr = persist.tile([1, 512], F32R, tag="onesr")
            nc.sync.dma_start(out=onesr[:], in_=onesd.ap())
            mask_sb = persist.tile([128, 256], BF, tag="mask")
            nc.sync.dma_start(out=mask_sb[:], in_=mask2.ap())
            wo_sb = persist.tile([128, 2, 1024], BF, tag="wo")
            nc.sync.dma_start(out=wo_sb[:], in_=woT.ap())
            bc1_sb = persist.tile([128, 320], BF, tag="bc1")
            nc.sync.dma_start(out=bc1_sb[:], in_=bc1.ap())

            qT_sb = persist.tile([128, NH, S], BF, tag="qT")
            kT_sb = persist.tile([128, NH, S], BF, tag="kT")
            v_sb = persist.tile([128, 16, NH, 65], BF, tag="v")
            nc.vector.memset(v_sb[:, :, :, 64:65], 1.0)

            bones = persist.tile([128, 1], BF, tag="bones")
            nc.vector.memset(bones[:], 1.0)
            cmag = persist.tile([128, 512], U32, tag="cmag")
            nc.vector.memset(cmag[:], QMAGIC)
            cone = persist.tile([128, 512], U32, tag="cone")
            nc.vector.memset(cone[:], 1)

            # deferred emission callbacks (norm tail of chunk c-1 runs
            # interleaved into chunk c's projection phase)
            pend = []
            tail_state = {}

            def emit_perhead(c, h, o_ps, in_proj=False):
                """1/r recip -> bb broadcast matmuls -> combine into occ."""
                hg, hh = h // 2, h % 2
                occ = occ_t
                cpy = nc.scalar.copy if in_proj else nc.vector.tensor_copy
                rsum = npool.tile([1, 2, 512], F32, tag="rsum",
                                  name=f"rsum_{c}_{h}")
                cpy(rsum[:], o_ps[64:65, :, :])
                invf = npool.tile([1, 2, 512], F32, tag="invf",
                                  name=f"invf_{c}_{h}")
                nc.vector.reciprocal_approx_fast(
                    out=invf[:], in_=rsum[:])
                inv = npool.tile([1, 2, 512], BF, tag="inv",
                                 name=f"inv_{c}_{h}")
                cpy(inv[:], invf[:])
                if DBG and c == NCH - 1 and h == NH - 1:
                    nc.sync.dma_start(
                        out=dbg_inv.ap(),
                        in_=inv[:].rearrange("p a b -> p (a b)"))
                bb = psc.tile([128, 512], F32, tag="sc",
                              name=f"bb_{c}_{h}")
                nc.tensor.matmul(bb[0:64, :], lhsT=bc1_sb[0:1, 0:64],
                                 rhs=inv[:, 0, :], start=True, stop=True,
                                 tile_position=(0, 0))
                nc.tensor.matmul(bb[64:128, :],
                                 lhsT=bc1_sb[0:1, 64 + 64 * h:128 + 64 * h],
                                 rhs=inv[:, 1, :], start=True, stop=True,
                                 tile_position=(0, 64))
                bbs = npool.tile([128, 512], F32, tag="bbs",
                                 name=f"bbs_{c}_{h}")
                cpy(bbs[:], bb[:])
                m1 = npool.tile([64, 512], F32, tag="m1", name=f"m1_{c}_{h}")
                nc.vector.tensor_mul(m1[:], o_ps[0:64, 0, :], bbs[0:64, :])
                m2t = npool.tile([64, 512], F32, tag="m2", name=f"m2_{c}_{h}")
                nc.vector.tensor_mul(m2t[:], o_ps[0:64, 1, :], bbs[64:128, :])
                nc.vector.tensor_add(occ[64 * hh:64 * hh + 64, hg, :],
                                     m1[:], m2t[:])

            def emit_tail_a(c):
                """sq -> ssq -> quake rsqrt (DVE heavy, no PE after ssq)."""
                occ = occ_t
                sq = npool.tile([128, 2, 512], BF, tag="sq",
                                name=f"sq_{c}")
                nc.vector.tensor_mul(sq[:], occ[:], occ[:])
                ssq = psc.tile([128, 512], F32, tag="sc", name=f"ssq_{c}")
                nc.vector.memset(ssq[:], 0.0)
                for h in range(4):
                    hg, hh = h // 2, h % 2
                    nc.tensor.matmul(
                        ssq[32 * h:32 * h + 1, :],
                        lhsT=bones[64 * hh:64 * hh + 64, 0:1],
                        rhs=sq[64 * hh:64 * hh + 64, hg, :],
                        start=True, stop=True,
                        tile_position=(64 * hh, 32 * h))
                m2q = npool.tile([128, 512], F32, tag="m2q", name=f"m2q_{c}")
                nc.vector.tensor_scalar(m2q[:], ssq[:], 1.0 / 64.0, EPS,
                                        op0=ALU.mult, op1=ALU.add)
                # quake rsqrt: y0 = bits((2C+1 - m) >> 1); 1 newton step
                q1 = npool.tile([128, 512], U32, tag="q1", name=f"q1_{c}")
                nc.vector.tensor_tensor(q1[:], cmag[:],
                                        m2q[:].bitcast(U32), op=ALU.subtract)
                nc.vector.tensor_tensor(q1[:], q1[:], cone[:],
                                        op=ALU.logical_shift_right)
                y0 = q1[:].bitcast(F32)
                uq = npool.tile([128, 512], F32, tag="uq", name=f"uq_{c}")
                nc.vector.tensor_mul(uq[:], y0, y0)
                wq_ = npool.tile([128, 512], F32, tag="wq_", name=f"wq_{c}")
                nc.vector.scalar_tensor_tensor(wq_[:], uq[:], -0.5, m2q[:],
                                               op0=ALU.mult, op1=ALU.mult)
                y2 = npool.tile([128, 512], F32, tag="y2", name=f"y2_{c}")
                nc.vector.scalar_tensor_tensor(y2[:], wq_[:], 1.5, y0,
                                               op0=ALU.add, op1=ALU.mult)
                # second newton step for accuracy
                nc.vector.tensor_mul(uq[:], y2[:], y2[:])
                nc.vector.scalar_tensor_tensor(wq_[:], uq[:], -0.5, m2q[:],
                                               op0=ALU.mult, op1=ALU.mult)
                y3f = npool.tile([128, 512], F32, tag="y3f", name=f"y3f_{c}")
                nc.vector.scalar_tensor_tensor(y3f[:], wq_[:], 1.5, y2[:],
                                               op0=ALU.add, op1=ALU.mult)
                y3 = npool.tile([128, 512], BF, tag="y3", name=f"y3_{c}")
                nc.scalar.copy(y3[:], y3f[:])
                if DBG and c == NCH - 1:
                    nc.sync.dma_start(out=dbg_y3.ap(), in_=y3[:])
                tail_state[c] = y3

            def emit_tail_b(c):
                """rms bcast matmuls + of mul."""
                occ = occ_t
                y3 = tail_state.pop(c)
                of = ofpool.tile([128, 2, 512], BF, tag="of", name=f"of_{c}")
                tail_state[(c, "of")] = of
                for hg in range(2):
                    rms = psc.tile([128, 512], F32, tag="sc",
                                   name=f"rms_{c}_{hg}")
                    nc.tensor.matmul(rms[0:64, :],
                                     lhsT=bc1_sb[64 * hg:64 * hg + 1, 0:64],
                                     rhs=y3[64 * hg:64 * hg + 1, :],
                                     start=True, stop=True,
                                     tile_position=(64 * hg, 0))
                    nc.tensor.matmul(rms[64:128, :],
                                     lhsT=bc1_sb[64 * hg + 32:64 * hg + 33,
                                                 0:64],
                                     rhs=y3[64 * hg + 32:64 * hg + 33, :],
                                     start=True, stop=True,
                                     tile_position=(64 * hg + 32, 64))
                    nc.vector.tensor_mul(of[:, hg, :], occ[:, hg, :], rms[:])

            def make_tail_c(c):
                """per-(t,n) out-projection closures, injected as PE filler
                into attention rounds."""
                cs = c * CH

                def one(t, n):
                    of = tail_state[(c, "of")]
                    yp = psc.tile([128, 512], F32, tag="sc",
                                  name=f"yp_{c}_{t}_{n}")
                    for kt in range(2):
                        nc.tensor.matmul(
                            yp[:], lhsT=of[:, kt, t * 128:t * 128 + 128],
                            rhs=wo_sb[:, kt, n * 512:n * 512 + 512],
                            start=(kt == 0), stop=(kt == 1))
                    ys = ypool.tile([128, 512], F32, tag="ys",
                                    name=f"ys_{c}_{t}_{n}")
                    nc.vector.tensor_copy(ys[:], yp[:])
                    nc.sync.dma_start(
                        out=y_out.ap()[cs + t * 128:cs + t * 128 + 128,
                                       n * 512:n * 512 + 512],
                        in_=ys[:])

                return [lambda t=t, n=n: one(t, n)
                        for t in range(4) for n in range(2)]

            ofpool = ctx.enter_context(tc.tile_pool(name="of", bufs=1))
            occ_t = persist.tile([128, 2, 512], F32, tag="occ",
                                 name="occ_t")

            for c in range(NCH):
                cs = c * CH
                # ======== projections for this chunk ========
                if c == 0:
                    xc = xc0
                else:
                    xc = xpool.tile([128, 8, CH], F32R, tag="xc",
                                    name=f"xc_{c}")
                    nc.sync.dma_start(out=xc[:], in_=xT_r[:, :, cs:cs + CH])

                def rope_stage(f, c=c, cs=cs):
                    """permute matmul + rope combine for feature tile f."""
                    qc, ps = proj_state[f]
                    qs_ps = psc.tile([128, 512], F32, tag="sc",
                                     name=f"qs_{c}_{f}")
                    nc.tensor.matmul(qs_ps[:], lhsT=pm_sb[:], rhs=qc[:],
                                     start=True, stop=True)
                    qs = rpool.tile([128, CH], BF, tag="qs",
                                    name=f"qsb_{c}_{f}")
                    nc.scalar.copy(qs[:], qs_ps[:])
                    t1 = rpool.tile([128, CH], BF, tag="t1",
                                    name=f"t1_{c}_{f}")
                    nc.vector.tensor_mul(t1[:], qc[:], rA_sb[:, cs:cs + CH])
                    t2 = rpool.tile([128, CH], BF, tag="t2",
                                    name=f"t2_{c}_{f}")
                    nc.vector.tensor_mul(t2[:], qs[:], rB_sb[:, cs:cs + CH])
                    fi = f % 4
                    dst = (qT_sb if f < 4 else kT_sb)[:, fi, cs:cs + CH]
                    nc.vector.tensor_add(dst, t1[:], t2[:])

                proj_state = {}
                for f in range(8):
                    ps = psc.tile([128, 512], F32, tag="sc",
                                  name=f"proj_{c}_{f}")
                    wsb = wq_sb if f < 4 else wk_sb
                    fi = f % 4
                    for kt in range(8):
                        nc.tensor.matmul(
                            ps[:], lhsT=wsb[:, kt, fi * 128:fi * 128 + 128],
                            rhs=xc[:, kt, :], start=(kt == 0), stop=(kt == 7))
                    qc = rpool.tile([128, CH], BF, tag="qc",
                                    name=f"qc_{c}_{f}")
                    nc.scalar.activation(qc[:], ps[:], AF.Identity,
                                         bias=bqp_sb[:, f:f + 1])
                    proj_state[f] = (qc, ps)
                    if f >= 1:
                        rope_stage(f - 1)
                    if pend:
                        stages = pend[0]
                        if f == 2 and "percb" in stages:
                            stages.pop("percb")()
                        elif f == 3 and "ta" in stages:
                            stages.pop("ta")()
                        elif f == 7 and "tb" in stages:
                            stages.pop("tb")()
                rope_stage(7)

                # v for the 4 key blocks of this chunk (seq-major)
                for m in range(4):
                    if m == 0 and pend and "tc" in pend[0]:
                        fillers.extend(pend[0].pop("tc")())
                        pend.pop(0)
                    vp = psc.tile([128, 512], F32, tag="sc",
                                  name=f"vp_{c}_{m}")
                    for kt in range(8):
                        nc.tensor.matmul(
                            vp[:, 0:256], lhsT=xc[:, kt, m * 128:m * 128 + 128],
                            rhs=wv_sb[:, kt, :], start=(kt == 0), stop=False)
                    nc.tensor.matmul(
                        vp[:, 0:256], lhsT=onesr[:, 0:128], rhs=bv_sb[:],
                        start=False, stop=True)
                    nc.scalar.copy(
                        v_sb[:, 4 * c + m, :, 0:64],
                        vp[:, 0:256].rearrange("p (h d) -> p h d", h=4))

                # ======== attention for this chunk ========
                njb = 4 * c + 4
                percbs = []
                if c == 0:
                    fillers = []
                for h in range(NH):
                    o_ps = psum.tile([128, 2, 512], F32, tag="o",
                                     name=f"o_{c}_{h}")
                    prevq = []
                    for jb in range(njb):
                        i0 = max(0, (jb - 4 * c) * 128)
                        at = apool.tile([128, 2, 512], BF, tag="at",
                                        name=f"at_{c}_{h}_{jb}")
                        for comp in range(2):
                            scb = psc.tile([128, 512], F32, tag="sc",
                                           name=f"sc_{c}_{h}_{jb}_{comp}")
                            nc.tensor.matmul(
                                scb[:, i0:512],
                                lhsT=kT_sb[64 * comp:64 * comp + 64, h,
                                           jb * JB:jb * JB + JB],
                                rhs=qT_sb[64 * comp:64 * comp + 64, h,
                                          cs + i0:cs + CH],
                                start=True, stop=True,
                                tile_position=(64 * comp, 0))
                            nc.scalar.activation(at[:, comp, i0:512],
                                                 scb[:, i0:512], AF.Exp,
                                                 scale=0.125)
                        if jb >= 4 * c:
                            nc.vector.tensor_mul(at[:, :, i0:i0 + 128],
                                                 at[:, :, i0:i0 + 128],
                                                 mask_sb[:])
                        pjb = 3 if njb == 4 else 4
                        if jb == pjb and percbs:
                            percbs.pop(0)()
                        elif jb != pjb and fillers:
                            fillers.pop(0)()
                        prevq.append((at, i0, jb))
                        if len(prevq) > 2:
                            pat, pi0, pjb = prevq.pop(0)
                            for comp in range(2):
                                nc.tensor.matmul(
                                    o_ps[0:65, comp, pi0:512],
                                    lhsT=v_sb[:, pjb, h, :],
                                    rhs=pat[:, comp, pi0:512],
                                    start=(pjb == 0), stop=(pjb == njb - 1))
                    for pat, pi0, pjb in prevq:
                        for comp in range(2):
                            nc.tensor.matmul(
                                o_ps[0:65, comp, pi0:512],
                                lhsT=v_sb[:, pjb, h, :],
                                rhs=pat[:, comp, pi0:512],
                                start=(pjb == 0), stop=(pjb == njb - 1))
                    percbs.append(
                        lambda c=c, h=h, o_ps=o_ps, ip=(h == NH - 1):
                        emit_perhead(c, h, o_ps, in_proj=ip))

                # last head's norm + chunk tail deferred into next chunk's
                # projection phase (flushed at staged f positions)
                pend.append({
                    "percb": percbs.pop(0),
                    "ta": lambda c=c: emit_tail_a(c),
                    "tb": lambda c=c: emit_tail_b(c),
                    "tc": lambda c=c: make_tail_c(c),
                })

            for stages in pend:
                for k in ("percb", "ta", "tb"):
                    if k in stages:
                        stages[k]()
                if "tc" in stages:
                    for cb in stages["tc"]():
                        cb()
            pend.clear()
            if DBG:
                nc.sync.dma_start(out=dbg_q.ap(),
                                  in_=qT_sb[:].rearrange("p a b -> p (a b)"))
                nc.sync.dma_start(out=dbg_k.ap(),
                                  in_=kT_sb[:].rearrange("p a b -> p (a b)"))
                nc.sync.dma_start(out=dbg_v.ap(),
                                  in_=v_sb[:].rearrange("p a b c -> p (a b c)"))
                nc.sync.dma_start(out=dbg_occ.ap(),
                                  in_=occ_t[:].rearrange("p a b -> p (a b)"))

    nc.compile()
    return nc


def _prep_inputs(x, Wq, bq, Wk, bk, Wv, bv, Wo, bo, head_norm_w,
                 lq1, lk1, lq2, lk2):
    lam_full = (LAMBDA_INIT
                + np.exp(np.sum(lq1 * lk1, -1))
                - np.exp(np.sum(lq2 * lk2, -1)))  # [H]

    half = HALF
    inv_freq = 1.0 / (10000.0 ** (np.arange(half, dtype=np.float64) / half))
    ang = np.arange(S, dtype=np.float64)[:, None] * inv_freq[None, :]  # [S,32]
    cosT = np.cos(ang).T.astype(np.float32)  # [32, S]
    sinT = np.sin(ang).T.astype(np.float32)
    ropeA = np.tile(cosT, (4, 1)).astype(BF16)                      # [128,S]
    ropeB = np.concatenate([-sinT, sinT, -sinT, sinT], 0).astype(BF16)

    mask1 = np.triu(np.ones((128, 128), np.float32))                # j<=i
    mask2 = np.concatenate([mask1, mask1], 1).astype(BF16)          # [128,256]

    swap_src = [32, 0, 96, 64]
    pm = np.zeros((128, 128), np.float32)
    for m in range(128):
        pm[swap_src[m // 32] + m % 32, m] = 1.0
    pm = pm.astype(BF16)

    in_maps = []
    for c in range(NCORES):
        b = c // 4
        h0 = 4 * (c % 4)
        rq = slice(h0 * 128, h0 * 128 + 512)
        rv = slice(h0 * 64, h0 * 64 + 256)

        xTc = np.ascontiguousarray(x[b].T)                          # [D, S]
        wq_l = Wq[rq].T  # [1024, 512]
        wk_l = Wk[rq].T
        wv_l = Wv[rv].T  # [1024, 256]
        wqr = np.ascontiguousarray(
            wq_l.reshape(8, 128, 512).transpose(1, 0, 2))
        wkr = np.ascontiguousarray(
            wk_l.reshape(8, 128, 512).transpose(1, 0, 2))
        wvr = np.ascontiguousarray(
            wv_l.reshape(8, 128, 256).transpose(1, 0, 2))

        hnw = head_norm_w[h0:h0 + 4].reshape(256)                   # local dims
        wo_l = Wo[:, rv].T * (hnw * (1.0 - LAMBDA_INIT))[:, None]   # [256,1024]
        wor = np.ascontiguousarray(
            wo_l.reshape(2, 128, 1024).transpose(1, 0, 2)).astype(BF16)

        # per-partition bias columns: col f = bias of feature tile f
        bqp_arr = np.stack([bq[rq][f * 128:f * 128 + 128] if f < 4
                            else bk[rq][(f - 4) * 128:(f - 4) * 128 + 128]
                            for f in range(8)], 1).astype(np.float32)  # [128,8]
        bv_arr = bv[rv].reshape(1, 256).astype(np.float32)

        # bc1: [1, 320]: cols 0:64 ones; cols 64+64h.. = -lambda_h
        bc1 = np.ones((1, 320), np.float32)
        for hl in range(4):
            bc1[0, 64 + 64 * hl:128 + 64 * hl] = -lam_full[h0 + hl]
        bc1 = np.tile(bc1, (128, 1))

        in_maps.append({
            "xT": xTc.astype(np.float32),
            "wqT": wqr.astype(np.float32),
            "wkT": wkr.astype(np.float32),
            "wvT": wvr.astype(np.float32),
            "bqp": bqp_arr,
            "bv": bv_arr,
            "woT": wor,
            "ropeA": ropeA,
            "ropeB": ropeB,
            "mask2": mask2,
            "pmat": pm,
            "bc1": bc1.astype(BF16),
            "onesd": np.ones((1, 512), np.float32),
        })
    return in_maps


def kernel(**inputs):
    from concourse.bass_utils import run_bass_kernel_spmd

    if "nc" not in _cache:
        _cache["nc"] = _build()
    nc = _cache["nc"]

    inputs = {k: np.asarray(v) for k, v in inputs.items()}
    in_maps = _prep_inputs(**inputs)
    res = run_bass_kernel_spmd(nc, in_maps, list(range(NCORES)))

    bo = inputs["bo"]
    y = np.zeros((B, S, D), np.float32)
    for b in range(B):
        acc = np.zeros((S, D), np.float32)
        for c in range(4 * b, 4 * b + 4):
            acc += res.results[c]["y"]
        y[b] = acc + bo[None, :]
    return y


# revision 39
# speedup vs baseline: 1.0319x; 1.0319x over previous
"""Differential self-attention on 8 Trainium2 NeuronCores.

Sharding: batch x head-group. Core c handles batch b = c//4 and heads
hs = 4*(c%4) .. 4*(c%4)+4 (4 of 16 heads). Each core computes q/k/v
projections for its heads, RoPE, both causal softmax score matrices
(flash-style, unnormalized, row sums fused into the attn@v matmul via a
ones-column appended to V), the differential combination + RMS norm
(quake rsqrt on DVE), and a partial out-projection over its heads'
dims. Host sums the 4 partial y per batch and adds bo.

Per-core layouts (feature-major "T" = [feat, seq]):
  qT/kT  [128, 4, 2048]  rows 0:64 = component 1, 64:128 = component 2
  v      [128, 16, 4, 65] (key-block, head, hd + ones col for row sums)
  scores sT [j=128, i<=512] so attn@v contracts j on partitions.

Row broadcasts (1/r, rsqrt(ms)) are PE matmuls with ones/(-lambda)
lhsT; no DRAM round-trips, no Ln activations (single act table load).
"""

import math

import numpy as np
import ml_dtypes

B, S, D = 2, 2048, 1024
H, HD = 16, 64
HALF = HD // 2
NCORES = 8
NH = 4            # heads per core
LAMBDA_INIT = 0.2
EPS = 1e-6
CH = 512          # query chunk
NCH = S // CH
JB = 128          # key block
BF16 = ml_dtypes.bfloat16

_cache = {}


def _build():
    import concourse.bass as bass
    import concourse.tile as tile
    from concourse import bacc, mybir

    F32R = mybir.dt.float32r
    F32 = mybir.dt.float32
    BF = mybir.dt.bfloat16
    U32 = mybir.dt.uint32
    AF = mybir.ActivationFunctionType
    ALU = mybir.AluOpType

    nc = bacc.Bacc("TRN2", debug=False, num_devices=NCORES)

    xT = nc.dram_tensor("xT", [D, S], F32R, kind="ExternalInput")
    wqT = nc.dram_tensor("wqT", [128, 8, 512], F32R, kind="ExternalInput")
    wkT = nc.dram_tensor("wkT", [128, 8, 512], F32R, kind="ExternalInput")
    wvT = nc.dram_tensor("wvT", [128, 8, 256], F32R, kind="ExternalInput")
    bqp = nc.dram_tensor("bqp", [128, 8], F32, kind="ExternalInput")
    bv = nc.dram_tensor("bv", [1, 256], F32R, kind="ExternalInput")
    woT = nc.dram_tensor("woT", [128, 2, 1024], BF, kind="ExternalInput")
    ropeA = nc.dram_tensor("ropeA", [128, S], BF, kind="ExternalInput")
    ropeB = nc.dram_tensor("ropeB", [128, S], BF, kind="ExternalInput")
    mask2 = nc.dram_tensor("mask2", [128, 256], BF, kind="ExternalInput")
    pmat = nc.dram_tensor("pmat", [128, 128], BF, kind="ExternalInput")
    bc1 = nc.dram_tensor("bc1", [128, 320], BF, kind="ExternalInput")
    onesd = nc.dram_tensor("onesd", [1, 512], F32R, kind="ExternalInput")
    y_out = nc.dram_tensor("y", [S, D], F32, kind="ExternalOutput")
    DBG = False
    if DBG:
        dbg_q = nc.dram_tensor("dbg_q", [128, NH * S], mybir.dt.bfloat16,
                               kind="ExternalOutput")
        dbg_k = nc.dram_tensor("dbg_k", [128, NH * S], mybir.dt.bfloat16,
                               kind="ExternalOutput")
        dbg_v = nc.dram_tensor("dbg_v", [128, 16 * NH * 65], mybir.dt.bfloat16,
                               kind="ExternalOutput")
        dbg_occ = nc.dram_tensor("dbg_occ", [128, 1024], F32,
                                 kind="ExternalOutput")
        dbg_y3 = nc.dram_tensor("dbg_y3", [128, 512], mybir.dt.bfloat16,
                                kind="ExternalOutput")
        dbg_inv = nc.dram_tensor("dbg_inv", [1, 1024], mybir.dt.bfloat16,
                                 kind="ExternalOutput")

    xT_r = xT.ap().rearrange("(t p) s -> p t s", p=128)
    QMAGIC = (0x5F3759DF * 2 + 1) & 0xFFFFFFFF

    with tile.TileContext(nc) as tc:
        import contextlib
        ctx = contextlib.ExitStack()
        with ctx:
            persist = ctx.enter_context(tc.tile_pool(name="persist", bufs=1))
            xpool = ctx.enter_context(tc.tile_pool(name="xc", bufs=2))
            rpool = ctx.enter_context(tc.tile_pool(name="rope", bufs=2))
            apool = ctx.enter_context(tc.tile_pool(name="atile", bufs=4))
            npool = ctx.enter_context(tc.tile_pool(name="norm", bufs=1))
            ypool = ctx.enter_context(tc.tile_pool(name="y", bufs=2))
            psum = ctx.enter_context(tc.tile_pool(name="ps", bufs=2, space="PSUM"))
            psc = ctx.enter_context(tc.tile_pool(name="psc", bufs=4, space="PSUM"))

            # ---- persistent tiles (DMA order: first-proj deps first)
            wq_sb = persist.tile([128, 8, 512], F32R, tag="wq")
            nc.sync.dma_start(out=wq_sb[:, 0:1, :], in_=wqT.ap()[:, 0:1, :])
            bqp_sb = persist.tile([128, 8], F32, tag="bqp")
            nc.sync.dma_start(out=bqp_sb[:], in_=bqp.ap())
            xc0 = xpool.tile([128, 8, CH], F32R, tag="xc", name="xc_pre0")
            nc.sync.dma_start(out=xc0[:, 0:1, :], in_=xT_r[:, 0:1, 0:CH])
            for hseg in range(1, 4):
                s0, s1 = 2 * hseg, 2 * hseg + 2
                nc.sync.dma_start(out=wq_sb[:, s0 - 1:s1 - 1, :],
                                  in_=wqT.ap()[:, s0 - 1:s1 - 1, :])
                nc.sync.dma_start(out=xc0[:, s0 - 1:s1 - 1, :],
                                  in_=xT_r[:, s0 - 1:s1 - 1, 0:CH])
            nc.sync.dma_start(out=wq_sb[:, 7:8, :], in_=wqT.ap()[:, 7:8, :])
            nc.sync.dma_start(out=xc0[:, 7:8, :], in_=xT_r[:, 7:8, 0:CH])
            rA_sb = persist.tile([128, S], BF, tag="ra")
            nc.sync.dma_start(out=rA_sb[:], in_=ropeA.ap())
            rB_sb = persist.tile([128, S], BF, tag="rb")
            nc.sync.dma_start(out=rB_sb[:], in_=ropeB.ap())
            pm_sb = persist.tile([128, 128], BF, tag="pm")
            nc.sync.dma_start(out=pm_sb[:], in_=pmat.ap())
            wk_sb = persist.tile([128, 8, 512], F32R, tag="wk")
            nc.sync.dma_start(out=wk_sb[:], in_=wkT.ap())
            wv_sb = persist.tile([128, 8, 256], F32R, tag="wv")
            nc.sync.dma_start(out=wv_sb[:], in_=wvT.ap())
            bv_sb = persist.tile([1, 256], F32R, tag="bv")
            nc.sync.dma_start(out=bv_sb[:], in_=bv.ap())
            onesr = persist.tile([1, 512], F32R, tag="onesr")
            nc.sync.dma_start(out=onesr[:], in_=onesd.ap())
            mask_sb = persist.tile([128, 256], BF, tag="mask")
            nc.sync.dma_start(out=mask_sb[:], in_=mask2.ap())
            wo_sb = persist.tile([128, 2, 1024], BF, tag="wo")
            nc.sync.dma_start(out=wo_sb[:], in_=woT.ap())
            bc1_sb = persist.tile([128, 320], BF, tag="bc1")
            nc.sync.dma_start(out=bc1_sb[:], in_=bc1.ap())

            qT_sb = persist.tile([128, NH, S], BF, tag="qT")
            kT_sb = persist.tile([128, NH, S], BF, tag="kT")
            v_sb = persist.tile([128, 16, NH, 65], BF, tag="v")
            nc.vector.memset(v_sb[:, :, :, 64:65], 1.0)

            bones = persist.tile([128, 1], BF, tag="bones")
            nc.vector.memset(bones[:], 1.0)
            cmag = persist.tile([128, 512], U32, tag="cmag")
            nc.vector.memset(cmag[:], QMAGIC)
            cone = persist.tile([128, 512], U32, tag="cone")
            nc.vector.memset(cone[:], 1)

            # deferred emission callbacks (norm tail of chunk c-1 runs
            # interleaved into chunk c's projection phase)
            pend = []
            tail_state = {}

            def emit_perhead(c, h, o_ps, in_proj=False):
                """1/r recip -> bb broadcast matmuls -> combine into occ."""
                hg, hh = h // 2, h % 2
                occ = occ_t
                cpy = nc.scalar.copy if in_proj else nc.vector.tensor_copy
                rsum = npool.tile([1, 2, 512], F32, tag="rsum",
                                  name=f"rsum_{c}_{h}")
                cpy(rsum[:], o_ps[64:65, :, :])
                invf = npool.tile([1, 2, 512], F32, tag="invf",
                                  name=f"invf_{c}_{h}")
                nc.vector.reciprocal_approx_fast(
                    out=invf[:], in_=rsum[:])
                inv = npool.tile([1, 2, 512], BF, tag="inv",
                                 name=f"inv_{c}_{h}")
                cpy(inv[:], invf[:])
                if DBG and c == NCH - 1 and h == NH - 1:
                    nc.sync.dma_start(
                        out=dbg_inv.ap(),
                        in_=inv[:].rearrange("p a b -> p (a b)"))
                bb = psc.tile([128, 512], F32, tag="sc",
                              name=f"bb_{c}_{h}")
                nc.tensor.matmul(bb[0:64, :], lhsT=bc1_sb[0:1, 0:64],
                                 rhs=inv[:, 0, :], start=True, stop=True,
                                 tile_position=(0, 0))
                nc.tensor.matmul(bb[64:128, :],
                                 lhsT=bc1_sb[0:1, 64 + 64 * h:128 + 64 * h],
                                 rhs=inv[:, 1, :], start=True, stop=True,
                                 tile_position=(0, 64))
                bbs = npool.tile([128, 512], F32, tag="bbs",
                                 name=f"bbs_{c}_{h}")
                cpy(bbs[:], bb[:])
                m1 = npool.tile([64, 512], F32, tag="m1", name=f"m1_{c}_{h}")
                nc.vector.tensor_mul(m1[:], o_ps[0:64, 0, :], bbs[0:64, :])
                m2t = npool.tile([64, 512], F32, tag="m2", name=f"m2_{c}_{h}")
                nc.vector.tensor_mul(m2t[:], o_ps[0:64, 1, :], bbs[64:128, :])
                nc.vector.tensor_add(occ[64 * hh:64 * hh + 64, hg, :],
                                     m1[:], m2t[:])

            def emit_tail_a(c):
                """sq -> ssq -> quake rsqrt (DVE heavy, no PE after ssq)."""
                occ = occ_t
                sq = npool.tile([128, 2, 512], BF, tag="sq",
                                name=f"sq_{c}")
                nc.vector.tensor_mul(sq[:], occ[:], occ[:])
                ssq = psc.tile([128, 512], F32, tag="sc", name=f"ssq_{c}")
                nc.vector.memset(ssq[:], 0.0)
                for h in range(4):
                    hg, hh = h // 2, h % 2
                    nc.tensor.matmul(
                        ssq[32 * h:32 * h + 1, :],
                        lhsT=bones[64 * hh:64 * hh + 64, 0:1],
                        rhs=sq[64 * hh:64 * hh + 64, hg, :],
                        start=True, stop=True,
                        tile_position=(64 * hh, 32 * h))
                m2q = npool.tile([128, 512], F32, tag="m2q", name=f"m2q_{c}")
                nc.vector.tensor_scalar(m2q[:], ssq[:], 1.0 / 64.0, EPS,
                                        op0=ALU.mult, op1=ALU.add)
                # quake rsqrt: y0 = bits((2C+1 - m) >> 1); 1 newton step
                q1 = npool.tile([128, 512], U32, tag="q1", name=f"q1_{c}")
                nc.vector.tensor_tensor(q1[:], cmag[:],
                                        m2q[:].bitcast(U32), op=ALU.subtract)
                nc.vector.tensor_tensor(q1[:], q1[:], cone[:],
                                        op=ALU.logical_shift_right)
                y0 = q1[:].bitcast(F32)
                uq = npool.tile([128, 512], F32, tag="uq", name=f"uq_{c}")
                nc.vector.tensor_mul(uq[:], y0, y0)
                wq_ = npool.tile([128, 512], F32, tag="wq_", name=f"wq_{c}")
                nc.vector.scalar_tensor_tensor(wq_[:], uq[:], -0.5, m2q[:],
                                               op0=ALU.mult, op1=ALU.mult)
                y2 = npool.tile([128, 512], F32, tag="y2", name=f"y2_{c}")
                nc.vector.scalar_tensor_tensor(y2[:], wq_[:], 1.5, y0,
                                               op0=ALU.add, op1=ALU.mult)
                # second newton step for accuracy
                nc.vector.tensor_mul(uq[:], y2[:], y2[:])
                nc.vector.scalar_tensor_tensor(wq_[:], uq[:], -0.5, m2q[:],
                                               op0=ALU.mult, op1=ALU.mult)
                y3f = npool.tile([128, 512], F32, tag="y3f", name=f"y3f_{c}")
                nc.vector.scalar_tensor_tensor(y3f[:], wq_[:], 1.5, y2[:],
                                               op0=ALU.add, op1=ALU.mult)
                y3 = npool.tile([128, 512], BF, tag="y3", name=f"y3_{c}")
                nc.scalar.copy(y3[:], y3f[:])
                if DBG and c == NCH - 1:
                    nc.sync.dma_start(out=dbg_y3.ap(), in_=y3[:])
                tail_state[c] = y3

            def emit_tail_b(c):
                """rms bcast matmuls + of mul."""
                occ = occ_t
                y3 = tail_state.pop(c)
                of = ofpool.tile([128, 2, 512], BF, tag="of", name=f"of_{c}")
                tail_state[(c, "of")] = of
                for hg in range(2):
                    rms = psc.tile([128, 512], F32, tag="sc",
                                   name=f"rms_{c}_{hg}")
                    nc.tensor.matmul(rms[0:64, :],
                                     lhsT=bc1_sb[64 * hg:64 * hg + 1, 0:64],
                                     rhs=y3[64 * hg:64 * hg + 1, :],
                                     start=True, stop=True,
                                     tile_position=(64 * hg, 0))
                    nc.tensor.matmul(rms[64:128, :],
                                     lhsT=bc1_sb[64 * hg + 32:64 * hg + 33,
                                                 0:64],
                                     rhs=y3[64 * hg + 32:64 * hg + 33, :],
                                     start=True, stop=True,
                                     tile_position=(64 * hg + 32, 64))
                    nc.vector.tensor_mul(of[:, hg, :], occ[:, hg, :], rms[:])

            def make_tail_c(c):
                """per-(t,n) out-projection closures, injected as PE filler
                into attention rounds."""
                cs = c * CH

                def one(t, n):
                    of = tail_state[(c, "of")]
                    yp = psc.tile([128, 512], F32, tag="sc",
                                  name=f"yp_{c}_{t}_{n}")
                    for kt in range(2):
                        nc.tensor.matmul(
                            yp[:], lhsT=of[:, kt, t * 128:t * 128 + 128],
                            rhs=wo_sb[:, kt, n * 512:n * 512 + 512],
                            start=(kt == 0), stop=(kt == 1))
                    ys = ypool.tile([128, 512], F32, tag="ys",
                                    name=f"ys_{c}_{t}_{n}")
                    nc.vector.tensor_copy(ys[:], yp[:])
                    nc.sync.dma_start(
                        out=y_out.ap()[cs + t * 128:cs + t * 128 + 128,
                                       n * 512:n * 512 + 512],
                        in_=ys[:])

                return [lambda t=t, n=n: one(t, n)
                        for t in range(4) for n in range(2)]

            ofpool = ctx.enter_context(tc.tile_pool(name="of", bufs=1))
            fillers = []
            occ_t = persist.tile([128, 2, 512], F32, tag="occ",
                                 name="occ_t")

            for c in range(NCH):
                cs = c * CH
                # ======== projections for this chunk ========
                if c == 0:
                    xc = xc0
                else:
                    xc = xpool.tile([128, 8, CH], F32R, tag="xc",
                                    name=f"xc_{c}")
                    nc.sync.dma_start(out=xc[:], in_=xT_r[:, :, cs:cs + CH])

                def rope_stage(f, c=c, cs=cs):
                    """permute matmul + rope combine for feature tile f."""
                    qc, ps = proj_state[f]
                    qs_ps = psc.tile([128, 512], F32, tag="sc",
                                     name=f"qs_{c}_{f}")
                    nc.tensor.matmul(qs_ps[:], lhsT=pm_sb[:], rhs=qc[:],
                                     start=True, stop=True)
                    qs = rpool.tile([128, CH], BF, tag="qs",
                                    name=f"qsb_{c}_{f}")
                    nc.scalar.copy(qs[:], qs_ps[:])
                    t1 = rpool.tile([128, CH], BF, tag="t1",
                                    name=f"t1_{c}_{f}")
                    nc.vector.tensor_mul(t1[:], qc[:], rA_sb[:, cs:cs + CH])
                    t2 = rpool.tile([128, CH], BF, tag="t2",
                                    name=f"t2_{c}_{f}")
                    nc.vector.tensor_mul(t2[:], qs[:], rB_sb[:, cs:cs + CH])
                    fi = f % 4
                    dst = (qT_sb if f < 4 else kT_sb)[:, fi, cs:cs + CH]
                    nc.vector.tensor_add(dst, t1[:], t2[:])

                while fillers:
                    fillers.pop(0)()
                proj_state = {}
                nproj = 8 if c == 0 else 4
                for f in range(nproj):
                    ps = psc.tile([128, 512], F32, tag="sc",
                                  name=f"proj_{c}_{f}")
                    wsb = wq_sb if f < 4 else wk_sb
                    fi = f % 4
                    for kt in range(8):
                        nc.tensor.matmul(
                            ps[:], lhsT=wsb[:, kt, fi * 128:fi * 128 + 128],
                            rhs=xc[:, kt, :], start=(kt == 0), stop=(kt == 7))
                    qc = rpool.tile([128, CH], BF, tag="qc",
                                    name=f"qc_{c}_{f}")
                    nc.scalar.activation(qc[:], ps[:], AF.Identity,
                                         bias=bqp_sb[:, f:f + 1])
                    proj_state[f] = (qc, ps)
                    if f >= 1:
                        rope_stage(f - 1)
                    if pend:
                        stages = pend[0]
                        if f == 1 and "percb" in stages:
                            stages.pop("percb")()
                        elif f == 2 and "ta" in stages:
                            stages.pop("ta")()
                if nproj:
                    rope_stage(nproj - 1)

                # v for the 4 key blocks of this chunk (seq-major)
                for m in range(4):
                    if pend:
                        stages = pend[0]
                        if m == 0 and "percb" in stages:
                            stages.pop("percb")()
                        elif m == 1 and "ta" in stages:
                            stages.pop("ta")()
                        elif m == 2 and "tb" in stages:
                            stages.pop("tb")()
                        elif m == 3 and "tc" in stages:
                            fillers.extend(stages.pop("tc")())
                            pend.pop(0)
                    vp = psc.tile([128, 512], F32, tag="sc",
                                  name=f"vp_{c}_{m}")
                    for kt in range(8):
                        nc.tensor.matmul(
                            vp[:, 0:256], lhsT=xc[:, kt, m * 128:m * 128 + 128],
                            rhs=wv_sb[:, kt, :], start=(kt == 0), stop=False)
                    nc.tensor.matmul(
                        vp[:, 0:256], lhsT=onesr[:, 0:128], rhs=bv_sb[:],
                        start=False, stop=True)
                    nc.scalar.copy(
                        v_sb[:, 4 * c + m, :, 0:64],
                        vp[:, 0:256].rearrange("p (h d) -> p h d", h=4))

                # ======== attention for this chunk ========
                njb = 4 * c + 4
                rctr = [0]
                percbs = []
                if c + 1 < NCH:
                    csn = (c + 1) * CH
                    xcn = xpool.tile([128, 8, CH], F32R, tag="xc",
                                     name=f"xc_{c + 1}")
                    nc.sync.dma_start(out=xcn[:], in_=xT_r[:, :, csn:csn + CH])

                    pstate = {}

                    def proj_a(f, cn=c + 1, xcn=xcn):
                        fi = f % 4
                        wsbj = wq_sb if f < 4 else wk_sb
                        pj = psc.tile([128, 512], F32, tag="sc",
                                      name=f"pj_{cn}_{f}")
                        for kt in range(8):
                            nc.tensor.matmul(
                                pj[:],
                                lhsT=wsbj[:, kt, fi * 128:fi * 128 + 128],
                                rhs=xcn[:, kt, :], start=(kt == 0),
                                stop=(kt == 7))
                        qcj = rpool.tile([128, CH], BF, tag="qc",
                                         name=f"qcj_{cn}_{f}")
                        nc.vector.tensor_scalar(qcj[:], pj[:],
                                                bqp_sb[:, f:f + 1], None,
                                                op0=ALU.add)
                        pstate[f] = qcj

                    def proj_b(f, cn=c + 1, csn=csn):
                        qcj = pstate.pop(f)
                        qsj_ps = psc.tile([128, 512], F32, tag="sc",
                                          name=f"qsj_{cn}_{f}")
                        nc.tensor.matmul(qsj_ps[:], lhsT=pm_sb[:],
                                         rhs=qcj[:], start=True, stop=True)
                        qsj = rpool.tile([128, CH], BF, tag="qs",
                                         name=f"qsj2_{cn}_{f}")
                        nc.vector.tensor_copy(qsj[:], qsj_ps[:])
                        t1j = rpool.tile([128, CH], BF, tag="t1",
                                         name=f"t1j_{cn}_{f}")
                        nc.vector.tensor_mul(t1j[:], qcj[:],
                                             rA_sb[:, csn:csn + CH])
                        t2j = rpool.tile([128, CH], BF, tag="t2",
                                         name=f"t2j_{cn}_{f}")
                        nc.vector.tensor_mul(t2j[:], qsj[:],
                                             rB_sb[:, csn:csn + CH])
                        dstj = (qT_sb if f < 4 else kT_sb)[:, f % 4,
                                                            csn:csn + CH]
                        nc.vector.tensor_add(dstj, t1j[:], t2j[:])

                    for f in range(4, 8):
                        fillers.append(lambda f=f: proj_a(f))
                        fillers.append(lambda f=f: proj_b(f))
                for h in range(NH):
                    o_ps = psum.tile([128, 2, 512], F32, tag="o",
                                     name=f"o_{c}_{h}")
                    prevq = []
                    for jb in range(njb):
                        i0 = max(0, (jb - 4 * c) * 128)
                        at = apool.tile([128, 2, 512], BF, tag="at",
                                        name=f"at_{c}_{h}_{jb}")
                        for comp in range(2):
                            scb = psc.tile([128, 512], F32, tag="sc",
                                           name=f"sc_{c}_{h}_{jb}_{comp}")
                            nc.tensor.matmul(
                                scb[:, i0:512],
                                lhsT=kT_sb[64 * comp:64 * comp + 64, h,
                                           jb * JB:jb * JB + JB],
                                rhs=qT_sb[64 * comp:64 * comp + 64, h,
                                          cs + i0:cs + CH],
                                start=True, stop=True,
                                tile_position=(64 * comp, 0))
                            nc.scalar.activation(at[:, comp, i0:512],
                                                 scb[:, i0:512], AF.Exp,
                                                 scale=0.125)
                        if jb >= 4 * c:
                            nc.vector.tensor_mul(at[:, :, i0:i0 + 128],
                                                 at[:, :, i0:i0 + 128],
                                                 mask_sb[:])
                        pjb = 3 if njb == 4 else 4
                        rctr[0] += 1
                        if jb == pjb and percbs:
                            percbs.pop(0)()
                        elif jb != pjb and fillers and rctr[0] % 2 == 0:
                            fillers.pop(0)()
                        prevq.append((at, i0, jb))
                        if len(prevq) > 2:
                            pat, pi0, pjb = prevq.pop(0)
                            for comp in range(2):
                                nc.tensor.matmul(
                                    o_ps[0:65, comp, pi0:512],
                                    lhsT=v_sb[:, pjb, h, :],
                                    rhs=pat[:, comp, pi0:512],
                                    start=(pjb == 0), stop=(pjb == njb - 1))
                    for pat, pi0, pjb in prevq:
                        for comp in range(2):
                            nc.tensor.matmul(
                                o_ps[0:65, comp, pi0:512],
                                lhsT=v_sb[:, pjb, h, :],
                                rhs=pat[:, comp, pi0:512],
                                start=(pjb == 0), stop=(pjb == njb - 1))
                    percbs.append(
                        lambda c=c, h=h, o_ps=o_ps, ip=(h == NH - 1):
                        emit_perhead(c, h, o_ps, in_proj=ip))

                # last head's norm + chunk tail deferred into next chunk's
                # projection phase (flushed at staged f positions)
                pend.append({
                    "percb": percbs.pop(0),
                    "ta": lambda c=c: emit_tail_a(c),
                    "tb": lambda c=c: emit_tail_b(c),
                    "tc": lambda c=c: make_tail_c(c),
                })

            while fillers:
                fillers.pop(0)()
            for stages in pend:
                for k in ("percb", "ta", "tb"):
                    if k in stages:
                        stages[k]()
                if "tc" in stages:
                    for cb in stages["tc"]():
                        cb()
            pend.clear()
            if DBG:
                nc.sync.dma_start(out=dbg_q.ap(),
                                  in_=qT_sb[:].rearrange("p a b -> p (a b)"))
                nc.sync.dma_start(out=dbg_k.ap(),
                                  in_=kT_sb[:].rearrange("p a b -> p (a b)"))
                nc.sync.dma_start(out=dbg_v.ap(),
                                  in_=v_sb[:].rearrange("p a b c -> p (a b c)"))
                nc.sync.dma_start(out=dbg_occ.ap(),
                                  in_=occ_t[:].rearrange("p a b -> p (a b)"))

    nc.compile()
    return nc


def _prep_inputs(x, Wq, bq, Wk, bk, Wv, bv, Wo, bo, head_norm_w,
                 lq1, lk1, lq2, lk2):
    lam_full = (LAMBDA_INIT
                + np.exp(np.sum(lq1 * lk1, -1))
                - np.exp(np.sum(lq2 * lk2, -1)))  # [H]

    half = HALF
    inv_freq = 1.0 / (10000.0 ** (np.arange(half, dtype=np.float64) / half))
    ang = np.arange(S, dtype=np.float64)[:, None] * inv_freq[None, :]  # [S,32]
    cosT = np.cos(ang).T.astype(np.float32)  # [32, S]
    sinT = np.sin(ang).T.astype(np.float32)
    ropeA = np.tile(cosT, (4, 1)).astype(BF16)                      # [128,S]
    ropeB = np.concatenate([-sinT, sinT, -sinT, sinT], 0).astype(BF16)

    mask1 = np.triu(np.ones((128, 128), np.float32))                # j<=i
    mask2 = np.concatenate([mask1, mask1], 1).astype(BF16)          # [128,256]

    swap_src = [32, 0, 96, 64]
    pm = np.zeros((128, 128), np.float32)
    for m in range(128):
        pm[swap_src[m // 32] + m % 32, m] = 1.0
    pm = pm.astype(BF16)

    in_maps = []
    for c in range(NCORES):
        b = c // 4
        h0 = 4 * (c % 4)
        rq = slice(h0 * 128, h0 * 128 + 512)
        rv = slice(h0 * 64, h0 * 64 + 256)

        xTc = np.ascontiguousarray(x[b].T)                          # [D, S]
        wq_l = Wq[rq].T  # [1024, 512]
        wk_l = Wk[rq].T
        wv_l = Wv[rv].T  # [1024, 256]
        wqr = np.ascontiguousarray(
            wq_l.reshape(8, 128, 512).transpose(1, 0, 2))
        wkr = np.ascontiguousarray(
            wk_l.reshape(8, 128, 512).transpose(1, 0, 2))
        wvr = np.ascontiguousarray(
            wv_l.reshape(8, 128, 256).transpose(1, 0, 2))

        hnw = head_norm_w[h0:h0 + 4].reshape(256)                   # local dims
        wo_l = Wo[:, rv].T * (hnw * (1.0 - LAMBDA_INIT))[:, None]   # [256,1024]
        wor = np.ascontiguousarray(
            wo_l.reshape(2, 128, 1024).transpose(1, 0, 2)).astype(BF16)

        # per-partition bias columns: col f = bias of feature tile f
        bqp_arr = np.stack([bq[rq][f * 128:f * 128 + 128] if f < 4
                            else bk[rq][(f - 4) * 128:(f - 4) * 128 + 128]
                            for f in range(8)], 1).astype(np.float32)  # [128,8]
        bv_arr = bv[rv].reshape(1, 256).astype(np.float32)

        # bc1: [1, 320]: cols 0:64 ones; cols 64+64h.. = -lambda_h
        bc1 = np.ones((1, 320), np.float32)
        for hl in range(4):
            bc1[0, 64 + 64 * hl:128 + 64 * hl] = -lam_full[h0 + hl]
        bc1 = np.tile(bc1, (128, 1))

        in_maps.append({
            "xT": xTc.astype(np.float32),
            "wqT": wqr.astype(np.float32),
            "wkT": wkr.astype(np.float32),
            "wvT": wvr.astype(np.float32),
            "bqp": bqp_arr,
            "bv": bv_arr,
            "woT": wor,
            "ropeA": ropeA,
            "ropeB": ropeB,
            "mask2": mask2,
            "pmat": pm,
            "bc1": bc1.astype(BF16),
            "onesd": np.ones((1, 512), np.float32),
        })
    return in_maps


def kernel(**inputs):
    from concourse.bass_utils import run_bass_kernel_spmd

    if "nc" not in _cache:
        _cache["nc"] = _build()
    nc = _cache["nc"]

    inputs = {k: np.asarray(v) for k, v in inputs.items()}
    in_maps = _prep_inputs(**inputs)
    res = run_bass_kernel_spmd(nc, in_maps, list(range(NCORES)))

    bo = inputs["bo"]
    y = np.zeros((B, S, D), np.float32)
    for b in range(B):
        acc = np.zeros((S, D), np.float32)
        for c in range(4 * b, 4 * b + 4):
            acc += res.results[c]["y"]
        y[b] = acc + bo[None, :]
    return y
